# revision 18
# baseline (speedup 1.0000x reference)
"""Trainium2 Bass kernel for nn_CrossAtt (dual cross-attention + concat +
residual + 3x3 conv + BN + ReLU), data-parallel over (batch, row-group)
across 8 cores.

Sharding: core i -> batch b = i//4, row-group rg = i%4 (16 output rows).
Host pre-ROLLS each core's x1/x2 along the flattened HW axis so the core's
18-row extended query window (16 rows + 1 halo row each side) sits at a
FIXED column range [0, 1152) of the rolled buffer. Attention is invariant
to the consistent key permutation the roll induces; edge-wrap halo rows are
zeroed by the conv mask, exactly like the out-of-image rows they replace.

Numerics / engine assignment:
 - x, Wq/Wk/Wv, Wcat in bf16 (full PE rate at any free size; halves DMA;
   kills all fp32->fp32r rounding copies).
 - S = k^T q accumulated in fp32 PSUM; exp on ACT with scale=1/4 and
   bias=-2 (range guard), output E in fp8e4.
 - A@V and softmax denominator via fp8 DoubleRow matmuls (two 128-key
   chunks per instruction).
 - Softmax normalization: DVE reciprocal + GPSIMD partition_broadcast
   (no ones-matmul / ACT copy).
 - conv runs as 2-row output slabs interleaved between attention blocks;
   BN+ReLU fused on ACT.
"""

import sys

sys.path.insert(0, "/opt/trn_rl_repo")

from collections import deque

import numpy as np
import ml_dtypes

import concourse.bacc as bacc
import concourse.tile as tile
from concourse import mybir
from concourse.bass_utils import run_bass_kernel_spmd

F32 = mybir.dt.float32
BF16 = mybir.dt.bfloat16
FP8 = mybir.dt.float8e4
AF = mybir.ActivationFunctionType
ALU = mybir.AluOpType
DR = mybir.MatmulPerfMode.DoubleRowSwInterleave

B, C, H, W = 2, 256, 64, 64
NW = H * W  # 4096 key positions
RE = 18  # extended rows per core (16 + halo)
NE = RE * W  # 1152 query positions
D_QK, D_V = 16, 128
N_CORES = 8
BN_EPS = 1e-5
BW = 384  # query block width (3 blocks per branch)
NBLK = NE // BW

_PROG_CACHE: dict = {}


def _build_program(gamma: float):
    nc = bacc.Bacc("TRN2", target_bir_lowering=False, debug=False, num_devices=N_CORES)

    def din(name, shape, dt=F32):
        return nc.dram_tensor(name, shape, dt, kind="ExternalInput").ap()

    def dout(name, shape):
        return nc.dram_tensor(name, shape, F32, kind="ExternalOutput").ap()

    x1d = din("x1d", [C, NW], BF16)
    x2d = din("x2d", [C, NW], BF16)
    wqkvd = din("wqkvd", [128, 2, 160], BF16)
    wcatd = din("wcatd", [128, 2, 9, 256], BF16)
    maskd = din("maskd", [128, RE, W], BF16)
    smalld = din("smalld", [128, 8])
    o1 = dout("o1", [C, 1024])
    o2 = dout("o2", [C, 1024])
    feat = dout("feat", [C, 1024])

    with tile.TileContext(nc) as tc:
        with (
            tc.tile_pool(name="constp", bufs=1) as constp,
            tc.tile_pool(name="projp", bufs=1) as projp,
            tc.tile_pool(name="outp", bufs=1) as outp,
            tc.tile_pool(name="ep", bufs=4) as ep,
            tc.tile_pool(name="wkp", bufs=3) as wkp,
            tc.tile_pool(name="spool", bufs=2, space="PSUM") as spool,
            tc.tile_pool(name="app", bufs=1, space="PSUM") as app,
            tc.tile_pool(name="pjp", bufs=1, space="PSUM") as pjp,
        ):
            # ---- persistent SBUF tiles ----
            x1 = constp.tile([128, 2, NW], BF16, name="x1")
            x2 = constp.tile([128, 2, NW], BF16, name="x2")
            wqkv = constp.tile([128, 2, 160], BF16, name="wqkv")
            wcat = constp.tile([128, 2, 9, 256], BF16, name="wcat")
            mask = constp.tile([128, RE, W], BF16, name="mask")
            small = constp.tile([128, 8], F32, name="small")
            wqk = wqkv[:, :, 0:32]
            wv = wqkv[:, :, 32:160]
            bqk = small[0:16, 0:2]
            bvg = small[:, 2:3]
            bn = small[:, 3:7]
            ones_dr = constp.tile([128, 2, 128], FP8, name="ones_dr")
            negc = constp.tile([128, 1], F32, name="negc")

            k_r = [projp.tile([16, NW], BF16, name=f"k_r{i}") for i in range(2)]
            q_r = [projp.tile([16, NE], BF16, name=f"q_r{i}") for i in range(2)]
            vT = [projp.tile([128, 16, 128, 2], FP8, name=f"vT{i}") for i in range(2)]
            xb = [projp.tile([128, NE], BF16, name=f"xb{i}") for i in range(2)]
            out_e = [outp.tile([128, 2, NE], F32, name=f"out_e{i}") for i in range(2)]
            spad = outp.tile([128, 2, RE, W + 2], BF16, name="spad")

            xs = [x1, x2]  # self input per branch
            xo = [x2, x1]  # other input per branch

            # ---- DMA issue: sync queue carries cc0, scalar cc1; x2 leads ----
            nc.sync.dma_start(small[:], smalld[:])
            nc.scalar.dma_start(wqkv[:], wqkvd[:])
            QW = 1024
            xq_order = [(x2, x2d, 0), (x1, x1d, 0), (x2, x2d, 1), (x2, x2d, 2),
                        (x1, x1d, 1), (x2, x2d, 3), (x1, x1d, 2), (x1, x1d, 3)]
            for xt, xd, q in xq_order:
                nc.sync.dma_start(
                    xt[:, 0, QW * q : QW * q + QW], xd[0:128, QW * q : QW * q + QW]
                )
                nc.scalar.dma_start(
                    xt[:, 1, QW * q : QW * q + QW], xd[128:256, QW * q : QW * q + QW]
                )
            # late-needed tensors queued last so they can't delay the x loads
            nc.sync.dma_start(mask[:], maskd[:])
            nc.scalar.dma_start(wcat[:], wcatd[:])
            nc.gpsimd.memset(ones_dr[:], 1.0)
            nc.gpsimd.memset(negc[:], -3.5)
            nc.gpsimd.memset(spad[:], 0.0)

            # ---- emission helpers ----
            def emit_psk(br, nt):
                """k projection for 512 keys: k of branch br comes from x_other."""
                psk = pjp.tile([16, 512], F32, name="psk")
                c0 = nt * 512
                for cc in range(2):
                    nc.tensor.matmul(
                        psk[:],
                        wqk[:, cc, 16:32],
                        xo[br][:, cc, c0 : c0 + 512],
                        start=(cc == 0),
                        stop=(cc == 1),
                    )
                nc.vector.tensor_scalar_add(k_r[br][:, c0 : c0 + 512], psk[:], bqk[:, 1:2])

            def emit_psv(br, nt):
                """v projection for 4 key chunks -> vT[br][:, 4nt:4nt+4, :] fp8."""
                psv = pjp.tile([128, 512], F32, name="psv")
                for s4 in range(4):
                    p0 = nt * 512 + s4 * 128
                    for cc in range(2):
                        nc.tensor.matmul(
                            psv[:, s4 * 128 : s4 * 128 + 128],
                            xs[br][:, cc, p0 : p0 + 128],
                            wv[:, cc, :],
                            start=(cc == 0),
                            stop=(cc == 1),
                            skip_group_check=True,
                        )
                for h in range(2):
                    for sub in range(2):
                        nc.vector.tensor_copy(
                            vT[br][:, 2 * nt + h : 2 * nt + h + 1, :, sub : sub + 1],
                            psv[:, (2 * h + sub) * 128 : (2 * h + sub + 1) * 128],
                        )

            def emit_psq(br, blk):
                psq = pjp.tile([16, BW], F32, name="psq", tag="psk")
                q0 = blk * BW
                for cc in range(2):
                    nc.tensor.matmul(
                        psq[:],
                        wqk[:, cc, 0:16],
                        xs[br][:, cc, q0 : q0 + BW],
                        start=(cc == 0),
                        stop=(cc == 1),
                    )
                nc.vector.tensor_scalar_add(q_r[br][:, q0 : q0 + BW], psq[:], bqk[:, 0:1])

            def emit_xb(br):
                # x_self low channels + gamma*bv (residual+bias base for concat half)
                nc.vector.tensor_scalar_add(xb[br][:], xs[br][:, 0, 0:NE], bvg[:])

            # conv slabs: slab k covers output window rows {2k+1, 2k+2}.
            # Emitted as chunks of 6 matmuls so interleaving into the attention
            # pair stream never stalls the exp cadence.
            def conv_slab_thunks(k, oc):
                psy_box = {}

                def mm_chunk(ci):
                    def run():
                        if ci == 0:
                            psy_box["t"] = pjp.tile(
                                [128, 128], F32, name="psy",
                                tag=("psk" if oc == 0 else "psv"),
                            )
                        psy = psy_box["t"]
                        for t in range(3 * ci, 3 * ci + 3):
                            dy, dx = t // 3, t % 3
                            for cc in range(2):
                                nc.tensor.matmul(
                                    psy[:],
                                    wcat[:, cc, t, oc * 128 : oc * 128 + 128],
                                    spad[:, cc, 2 * k + dy : 2 * k + dy + 2, dx : dx + 64],
                                    start=(t == 0 and cc == 0),
                                    stop=(t == 8 and cc == 1),
                                    skip_group_check=True,
                                )
                    return run

                def finish():
                    psy = psy_box["t"]
                    fs = wkp.tile([128, 128], F32, name="fs")
                    nc.scalar.activation(
                        fs[:], psy[:], AF.Relu,
                        bias=bn[:, 2 + oc : 3 + oc], scale=bn[:, oc : oc + 1],
                    )
                    nc.sync.dma_start(
                        feat[128 * oc : 128 * oc + 128, 128 * k : 128 * k + 128], fs[:]
                    )

                return [mm_chunk(0), mm_chunk(1), mm_chunk(2), finish]

            def emit_conv_slab(k, oc):
                for th in conv_slab_thunks(k, oc):
                    th()

            def emit_spad_prep(b, half=None):
                """spad rows 6b..6b+6 = out1+out2 (edge rows pre-masked in
                out_e by emit_strip). half=0/1 emits 3-row halves."""
                rr = {None: (0, 6), 0: (0, 3), 1: (3, 6)}[half]
                for cc in range(2):
                    r0, q0 = 6 * b + rr[0], (6 * b + rr[0]) * W
                    n = (rr[1] - rr[0]) * W
                    nc.vector.tensor_add(
                        spad[:, cc, r0 : r0 + rr[1] - rr[0], 1:65],
                        out_e[0][:, cc, q0 : q0 + n],
                        out_e[1][:, cc, q0 : q0 + n],
                    )

            def emit_strip(br, row):
                # zero the wrap-garbage halo row of out_e (only rg edge cores
                # have a 0-row in mask; interior cores multiply by ones)
                q0 = row * W
                for cc in range(2):
                    nc.vector.tensor_tensor(
                        out_e[br][:, cc, q0 : q0 + W],
                        out_e[br][:, cc, q0 : q0 + W],
                        mask[:, row, :],
                        ALU.mult,
                    )

            def emit_postlude(br, blk, av, den):
                # normalization + residual/concat epilogue
                q0 = blk * BW
                rb = wkp.tile([128, BW], F32, name="rb")
                nc.vector.reciprocal(rb[:], den[:])
                tmp = wkp.tile([128, BW], F32, name="tmp")
                nc.vector.tensor_tensor(tmp[:], av[:], rb[:], ALU.mult)
                nc.vector.scalar_tensor_tensor(
                    out_e[br][:, 0, q0 : q0 + BW], tmp[:], gamma, xb[br][:, q0 : q0 + BW],
                    ALU.mult, ALU.add,
                )
                nc.vector.scalar_tensor_tensor(
                    out_e[br][:, 1, q0 : q0 + BW],
                    xs[br][:, 0, q0 : q0 + BW], gamma, xs[br][:, 1, q0 : q0 + BW],
                    ALU.mult, ALU.add,
                )

            def store_o(br, od):
                nc.sync.dma_start(od[0:128, :], out_e[br][:, 0, 64:1088])
                nc.sync.dma_start(od[128:256, :], out_e[br][:, 1, 64:1088])

            # ---- streaming pair pipeline across all 6 blocks ----
            ORDER = [(0, 0), (1, 0), (0, 1), (1, 1), (0, 2), (1, 2)]
            chase_map = {}  # (bi, p) -> thunks emitted before that pair's S
            post_map = {}  # (bi, p) -> thunks emitted right after that pair's exp
            for nt in range(8):
                chase_map[(0, 2 * nt)] = [lambda nt=nt: emit_psk(0, nt)]
                chase_map[(1, 2 * nt)] = [lambda nt=nt: emit_psk(1, nt)]
                post_map[(0, 2 * nt)] = [lambda nt=nt: emit_psv(0, nt)]
                post_map[(1, 2 * nt)] = [lambda nt=nt: emit_psv(1, nt)]
            chase_map[(0, 0)].append(lambda: emit_psq(0, 0))
            chase_map[(0, 2)].append(lambda: emit_psq(1, 0))
            post_map[(0, 8)].append(lambda: emit_xb(0))
            post_map[(0, 10)].append(lambda: emit_xb(1))
            chase_map[(1, 4)].append(lambda: emit_psq(0, 1))
            chase_map[(1, 6)].append(lambda: emit_psq(1, 1))

            between_map = {  # (bi, p) -> thunks emitted after that pair's pop
                (1, 2): [lambda: emit_strip(0, 0)],
                (2, 1): [lambda: emit_strip(1, 0), lambda: emit_spad_prep(0),
                         lambda: emit_psq(0, 2)],
                (3, 2): [lambda: emit_psq(1, 2)],
                (4, 1): [lambda: emit_spad_prep(1)],
                (5, 1): [lambda: emit_strip(0, 17)],
                (5, 2): [lambda: store_o(0, o1)],
            }
            # conv slabs 0..4 spread across blocks idx2-idx5 in 6-matmul chunks
            fill = []
            for k, oc in [(0, 0), (0, 1), (1, 0), (1, 1)]:
                fill += conv_slab_thunks(k, oc)
            for i, th in enumerate(fill):
                between_map.setdefault((2, 3 + (i * 12) // len(fill))
                                       if False else (2 + (3 + i) // 14, (3 + i) % 14),
                                       []).append(th)
            fill2 = []
            for k, oc in [(2, 0), (2, 1), (3, 0), (3, 1), (4, 0), (4, 1)]:
                fill2 += conv_slab_thunks(k, oc)
            for i, th in enumerate(fill2):
                between_map.setdefault((4 + (2 + i) // 14, (2 + i) % 14),
                                       []).append(th)

            block_acc = {}
            pend = deque()

            def pop_one():
                bi, br, p, E = pend.popleft()
                if p == 0:
                    block_acc[bi] = (
                        app.tile([128, BW], F32, name="av"),
                        app.tile([128, BW], F32, name="den"),
                    )
                av, den = block_acc[bi]
                nc.tensor.matmul(
                    av[:], vT[br][:, p : p + 1, :, :], E[:],
                    start=(p == 0), stop=(p == 15),
                    perf_mode=DR, skip_group_check=True,
                )
                nc.tensor.matmul(
                    den[:], ones_dr[:], E[:],
                    start=(p == 0), stop=(p == 15),
                    perf_mode=DR, skip_group_check=True,
                )
                if p == 15:
                    br_, blk_ = ORDER[bi]
                    emit_postlude(br_, blk_, av, den)

            for bi, (br, blk) in enumerate(ORDER):
                q0 = blk * BW
                for p in range(16):
                    for th in chase_map.get((bi, p), ()):
                        th()
                    s_t = spool.tile([128, 2, 512], F32, name="s_t")
                    for j in range(2):
                        nc.tensor.matmul(
                            s_t[:, j, 0:BW],
                            k_r[br][:, (2 * p + j) * 128 : (2 * p + j) * 128 + 128],
                            q_r[br][:, q0 : q0 + BW],
                            start=True,
                            stop=True,
                        )
                    E = ep.tile([128, 2, BW], FP8, name="E")
                    nc.scalar.activation(
                        E[:], s_t[:, :, 0:BW], AF.Exp, scale=0.25, bias=negc[:]
                    )
                    pend.append((bi, br, p, E))
                    for th in post_map.get((bi, p), ()):
                        th()
                    if len(pend) > 2:
                        pop_one()
                    for th in between_map.get((bi, p), ()):
                        th()
            while pend:
                pop_one()

            # ---- tail: last conv rows + stores ----
            emit_strip(1, 17)
            emit_spad_prep(2, half=0)
            store_o(1, o2)
            emit_conv_slab(5, 0)
            emit_spad_prep(2, half=1)
            emit_conv_slab(5, 1)
            for k in (6, 7):
                for oc in range(2):
                    emit_conv_slab(k, oc)

    nc.compile()
    return nc


def _prep_inputs(input1, input2, Wq, bq, Wk, bk, Wv, bv, gamma, Wcat, bn_gamma, bn_beta):
    f32 = np.float32
    bf16 = ml_dtypes.bfloat16
    g = f32(np.asarray(gamma).reshape(-1)[0])
    x1 = np.asarray(input1, f32).reshape(B, C, NW)
    x2 = np.asarray(input2, f32).reshape(B, C, NW)
    Wq, Wk, Wv = (np.asarray(w, f32) for w in (Wq, Wk, Wv))
    Wcat = np.asarray(Wcat, f32)

    wqkv = np.zeros((128, 2, 160), f32)
    for cc in range(2):
        wqkv[:, cc, 0:16] = Wq.T[128 * cc : 128 * cc + 128]
        wqkv[:, cc, 16:32] = Wk.T[128 * cc : 128 * cc + 128]
        # column-reversed for the DoubleRowSwInterleave weight layout
        wqkv[:, cc, 32:160] = Wv.T[128 * cc : 128 * cc + 128][:, ::-1]

    # [t, cin, cout]
    Wt = Wcat.transpose(2, 3, 1, 0).reshape(9, 256, 256)
    wcat2 = np.zeros((128, 2, 9, 256), f32)
    for cc in range(2):
        wcat2[:, cc] = Wt[:, 128 * cc : 128 * cc + 128, :].transpose(1, 0, 2)

    small = np.zeros((128, 8), f32)
    small[0:16, 0] = np.asarray(bq, f32)
    small[0:16, 1] = np.asarray(bk, f32)
    small[:, 2] = g * np.asarray(bv, f32)
    bnscale = (np.asarray(bn_gamma, f32) / np.sqrt(f32(1.0) + f32(BN_EPS))).astype(f32)
    bnb = np.asarray(bn_beta, f32)
    small[:, 3] = bnscale[0:128]
    small[:, 4] = bnscale[128:256]
    small[:, 5] = bnb[0:128]
    small[:, 6] = bnb[128:256]

    wqkv_b = wqkv.astype(bf16)
    wcat_b = wcat2.astype(bf16)

    in_maps = []
    for core in range(N_CORES):
        b, rg = core // 4, core % 4
        roll = (16 * rg - 1) * 64  # window col j = image pos (roll + j) mod NW

        rows = np.ones(RE, f32)
        if rg == 0:
            rows[0] = 0.0
        if rg == 3:
            rows[RE - 1] = 0.0
        msk = np.broadcast_to(
            np.repeat(rows, W).reshape(RE, W)[None], (128, RE, W)
        ).astype(bf16)

        in_maps.append(
            {
                "x1d": np.ascontiguousarray(np.roll(x1[b], -roll, axis=1)).astype(bf16),
                "x2d": np.ascontiguousarray(np.roll(x2[b], -roll, axis=1)).astype(bf16),
                "wqkvd": wqkv_b,
                "wcatd": wcat_b,
                "maskd": msk,
                "smalld": small,
            }
        )
    return in_maps


def _assemble(results):
    f32 = np.float32
    feat_sum = np.empty((B, C, H, W), f32)
    out1 = np.empty((B, C, H, W), f32)
    out2 = np.empty((B, C, H, W), f32)
    for core in range(N_CORES):
        b, rg = core // 4, core % 4
        r0 = 16 * rg
        r = results[core]
        out1[b, :, r0 : r0 + 16] = r["o1"].reshape(C, 16, W)
        out2[b, :, r0 : r0 + 16] = r["o2"].reshape(C, 16, W)
        feat_sum[b, :, r0 : r0 + 16] = r["feat"].reshape(C, 16, W)
    return feat_sum, out1, out2


def _get_program(gamma: float):
    if gamma not in _PROG_CACHE:
        _PROG_CACHE[gamma] = _build_program(gamma)
    return _PROG_CACHE[gamma]


def kernel(input1, input2, Wq, bq, Wk, bk, Wv, bv, gamma, Wcat, bn_gamma, bn_beta):
    g = float(np.asarray(gamma).reshape(-1)[0])
    nc = _get_program(g)
    in_maps = _prep_inputs(
        input1, input2, Wq, bq, Wk, bk, Wv, bv, gamma, Wcat, bn_gamma, bn_beta
    )
    res = run_bass_kernel_spmd(nc, in_maps, core_ids=list(range(N_CORES)))
    return _assemble(res.results)


def run_traced(inputs):
    """For test.py: run and return (outputs, exec_time_ns)."""
    g = float(np.asarray(inputs["gamma"]).reshape(-1)[0])
    nc = _get_program(g)
    in_maps = _prep_inputs(**inputs)
    res = run_bass_kernel_spmd(nc, in_maps, core_ids=list(range(N_CORES)))
    return _assemble(res.results), res.exec_time_ns


# revision 19
# speedup vs baseline: 1.0138x; 1.0138x over previous
"""Trainium2 Bass kernel for nn_CrossAtt (dual cross-attention + concat +
residual + 3x3 conv + BN + ReLU), data-parallel over (batch, row-group)
across 8 cores.

Sharding: core i -> batch b = i//4, row-group rg = i%4 (16 output rows).
Host pre-ROLLS each core's x1/x2 along the flattened HW axis so the core's
18-row extended query window (16 rows + 1 halo row each side) sits at a
FIXED column range [0, 1152) of the rolled buffer. Attention is invariant
to the consistent key permutation the roll induces; edge-wrap halo rows are
zeroed by the conv mask, exactly like the out-of-image rows they replace.

Numerics / engine assignment:
 - x, Wq/Wk/Wv, Wcat in bf16 (full PE rate at any free size; halves DMA;
   kills all fp32->fp32r rounding copies).
 - S = k^T q accumulated in fp32 PSUM; exp on ACT with scale=1/4 and
   bias=-2 (range guard), output E in fp8e4.
 - A@V and softmax denominator via fp8 DoubleRow matmuls (two 128-key
   chunks per instruction).
 - Softmax normalization: DVE reciprocal + GPSIMD partition_broadcast
   (no ones-matmul / ACT copy).
 - conv runs as 2-row output slabs interleaved between attention blocks;
   BN+ReLU fused on ACT.
"""

import sys

sys.path.insert(0, "/opt/trn_rl_repo")

from collections import deque

import numpy as np
import ml_dtypes

import concourse.bacc as bacc
import concourse.tile as tile
from concourse import mybir
from concourse.bass_utils import run_bass_kernel_spmd

F32 = mybir.dt.float32
BF16 = mybir.dt.bfloat16
FP8 = mybir.dt.float8e4
AF = mybir.ActivationFunctionType
ALU = mybir.AluOpType
DR = mybir.MatmulPerfMode.DoubleRowSwInterleave

B, C, H, W = 2, 256, 64, 64
NW = H * W  # 4096 key positions
RE = 18  # extended rows per core (16 + halo)
NE = RE * W  # 1152 query positions
D_QK, D_V = 16, 128
N_CORES = 8
BN_EPS = 1e-5
BW = 384  # query block width (3 blocks per branch)
NBLK = NE // BW

_PROG_CACHE: dict = {}


def _build_program(gamma: float):
    nc = bacc.Bacc("TRN2", target_bir_lowering=False, debug=False, num_devices=N_CORES)

    def din(name, shape, dt=F32):
        return nc.dram_tensor(name, shape, dt, kind="ExternalInput").ap()

    def dout(name, shape):
        return nc.dram_tensor(name, shape, F32, kind="ExternalOutput").ap()

    x1d = din("x1d", [C, NW], BF16)
    x2d = din("x2d", [C, NW], BF16)
    wqkvd = din("wqkvd", [128, 2, 160], BF16)
    wcatd = din("wcatd", [128, 2, 9, 256], BF16)
    maskd = din("maskd", [128, RE, W], BF16)
    smalld = din("smalld", [128, 8])
    o1 = dout("o1", [C, 1024])
    o2 = dout("o2", [C, 1024])
    feat = dout("feat", [C, 1024])

    with tile.TileContext(nc) as tc:
        with (
            tc.tile_pool(name="constp", bufs=1) as constp,
            tc.tile_pool(name="projp", bufs=1) as projp,
            tc.tile_pool(name="outp", bufs=1) as outp,
            tc.tile_pool(name="ep", bufs=4) as ep,
            tc.tile_pool(name="wkp", bufs=3) as wkp,
            tc.tile_pool(name="spool", bufs=2, space="PSUM") as spool,
            tc.tile_pool(name="app", bufs=1, space="PSUM") as app,
            tc.tile_pool(name="pjp", bufs=1, space="PSUM") as pjp,
        ):
            # ---- persistent SBUF tiles ----
            x1 = constp.tile([128, 2, NW], BF16, name="x1")
            x2 = constp.tile([128, 2, NW], BF16, name="x2")
            wqkv = constp.tile([128, 2, 160], BF16, name="wqkv")
            wcat = constp.tile([128, 2, 9, 256], BF16, name="wcat")
            mask = constp.tile([128, RE, W], BF16, name="mask")
            small = constp.tile([128, 8], F32, name="small")
            wqk = wqkv[:, :, 0:32]
            wv = wqkv[:, :, 32:160]
            bqk = small[0:16, 0:2]
            bvg = small[:, 2:3]
            bn = small[:, 3:7]
            ones_dr = constp.tile([128, 2, 128], FP8, name="ones_dr")
            negc = constp.tile([128, 1], F32, name="negc")

            k_r = [projp.tile([16, NW], BF16, name=f"k_r{i}") for i in range(2)]
            q_r = [projp.tile([16, NE], BF16, name=f"q_r{i}") for i in range(2)]
            vT = [projp.tile([128, 16, 128, 2], FP8, name=f"vT{i}") for i in range(2)]
            xb = [projp.tile([128, NE], BF16, name=f"xb{i}") for i in range(2)]
            out_e = [outp.tile([128, 2, NE], F32, name=f"out_e{i}") for i in range(2)]
            spad = outp.tile([128, 2, RE, W + 2], BF16, name="spad")

            xs = [x1, x2]  # self input per branch
            xo = [x2, x1]  # other input per branch

            # ---- DMA issue: sync queue carries cc0, scalar cc1; x2 leads ----
            nc.sync.dma_start(small[:], smalld[:])
            nc.scalar.dma_start(wqkv[:], wqkvd[:])
            QW = 1024
            xq_order = [(x2, x2d, 0), (x2, x2d, 1), (x1, x1d, 0), (x2, x2d, 2),
                        (x1, x1d, 1), (x2, x2d, 3), (x1, x1d, 2), (x1, x1d, 3)]
            for xt, xd, q in xq_order:
                nc.sync.dma_start(
                    xt[:, 0, QW * q : QW * q + QW], xd[0:128, QW * q : QW * q + QW]
                )
                nc.scalar.dma_start(
                    xt[:, 1, QW * q : QW * q + QW], xd[128:256, QW * q : QW * q + QW]
                )
            # late-needed tensors queued last so they can't delay the x loads
            nc.sync.dma_start(mask[:], maskd[:])
            nc.scalar.dma_start(wcat[:], wcatd[:])
            nc.gpsimd.memset(ones_dr[:], 1.0)
            nc.gpsimd.memset(negc[:], -3.5)
            nc.gpsimd.memset(spad[:], 0.0)

            # ---- emission helpers ----
            def emit_psk(br, nt):
                """k projection for 512 keys: k of branch br comes from x_other."""
                psk = pjp.tile([16, 512], F32, name="psk")
                c0 = nt * 512
                for cc in range(2):
                    nc.tensor.matmul(
                        psk[:],
                        wqk[:, cc, 16:32],
                        xo[br][:, cc, c0 : c0 + 512],
                        start=(cc == 0),
                        stop=(cc == 1),
                    )
                nc.vector.tensor_scalar_add(k_r[br][:, c0 : c0 + 512], psk[:], bqk[:, 1:2])

            def emit_psv(br, nt):
                """v projection for 4 key chunks -> vT[br][:, 4nt:4nt+4, :] fp8."""
                psv = pjp.tile([128, 512], F32, name="psv")
                for s4 in range(4):
                    p0 = nt * 512 + s4 * 128
                    for cc in range(2):
                        nc.tensor.matmul(
                            psv[:, s4 * 128 : s4 * 128 + 128],
                            xs[br][:, cc, p0 : p0 + 128],
                            wv[:, cc, :],
                            start=(cc == 0),
                            stop=(cc == 1),
                            skip_group_check=True,
                        )
                for h in range(2):
                    for sub in range(2):
                        nc.vector.tensor_copy(
                            vT[br][:, 2 * nt + h : 2 * nt + h + 1, :, sub : sub + 1],
                            psv[:, (2 * h + sub) * 128 : (2 * h + sub + 1) * 128],
                        )

            def emit_psq(br, blk):
                psq = pjp.tile([16, BW], F32, name="psq", tag="psk")
                q0 = blk * BW
                for cc in range(2):
                    nc.tensor.matmul(
                        psq[:],
                        wqk[:, cc, 0:16],
                        xs[br][:, cc, q0 : q0 + BW],
                        start=(cc == 0),
                        stop=(cc == 1),
                    )
                nc.vector.tensor_scalar_add(q_r[br][:, q0 : q0 + BW], psq[:], bqk[:, 0:1])

            def emit_xb(br):
                # x_self low channels + gamma*bv (residual+bias base for concat half)
                nc.vector.tensor_scalar_add(xb[br][:], xs[br][:, 0, 0:NE], bvg[:])

            # conv slabs: slab k covers output window rows {2k+1, 2k+2}.
            # Emitted as chunks of 6 matmuls so interleaving into the attention
            # pair stream never stalls the exp cadence.
            def conv_slab_thunks(k, oc):
                psy_box = {}

                def mm_chunk(ci):
                    def run():
                        if ci == 0:
                            psy_box["t"] = pjp.tile(
                                [128, 128], F32, name="psy",
                                tag=("psk" if oc == 0 else "psv"),
                            )
                        psy = psy_box["t"]
                        for t in range(3 * ci, 3 * ci + 3):
                            dy, dx = t // 3, t % 3
                            for cc in range(2):
                                nc.tensor.matmul(
                                    psy[:],
                                    wcat[:, cc, t, oc * 128 : oc * 128 + 128],
                                    spad[:, cc, 2 * k + dy : 2 * k + dy + 2, dx : dx + 64],
                                    start=(t == 0 and cc == 0),
                                    stop=(t == 8 and cc == 1),
                                    skip_group_check=True,
                                )
                    return run

                def finish():
                    psy = psy_box["t"]
                    fs = wkp.tile([128, 128], F32, name="fs")
                    nc.scalar.activation(
                        fs[:], psy[:], AF.Relu,
                        bias=bn[:, 2 + oc : 3 + oc], scale=bn[:, oc : oc + 1],
                    )
                    nc.sync.dma_start(
                        feat[128 * oc : 128 * oc + 128, 128 * k : 128 * k + 128], fs[:]
                    )

                return [mm_chunk(0), mm_chunk(1), mm_chunk(2), finish]

            def emit_conv_slab(k, oc):
                for th in conv_slab_thunks(k, oc):
                    th()

            def emit_spad_prep(b, half=None):
                """spad rows 6b..6b+6 = out1+out2 (edge rows pre-masked in
                out_e by emit_strip). half=0/1 emits 3-row halves."""
                rr = {None: (0, 6), 0: (0, 3), 1: (3, 6)}[half]
                for cc in range(2):
                    r0, q0 = 6 * b + rr[0], (6 * b + rr[0]) * W
                    n = (rr[1] - rr[0]) * W
                    nc.vector.tensor_add(
                        spad[:, cc, r0 : r0 + rr[1] - rr[0], 1:65],
                        out_e[0][:, cc, q0 : q0 + n],
                        out_e[1][:, cc, q0 : q0 + n],
                    )

            def emit_strip(br, row):
                # zero the wrap-garbage halo row of out_e (only rg edge cores
                # have a 0-row in mask; interior cores multiply by ones)
                q0 = row * W
                for cc in range(2):
                    nc.vector.tensor_tensor(
                        out_e[br][:, cc, q0 : q0 + W],
                        out_e[br][:, cc, q0 : q0 + W],
                        mask[:, row, :],
                        ALU.mult,
                    )

            def emit_postlude(br, blk, av, den):
                # normalization + residual/concat epilogue
                q0 = blk * BW
                rb = wkp.tile([128, BW], F32, name="rb")
                nc.vector.reciprocal(rb[:], den[:])
                tmp = wkp.tile([128, BW], F32, name="tmp")
                nc.vector.tensor_tensor(tmp[:], av[:], rb[:], ALU.mult)
                nc.vector.scalar_tensor_tensor(
                    out_e[br][:, 0, q0 : q0 + BW], tmp[:], gamma, xb[br][:, q0 : q0 + BW],
                    ALU.mult, ALU.add,
                )
                nc.vector.scalar_tensor_tensor(
                    out_e[br][:, 1, q0 : q0 + BW],
                    xs[br][:, 0, q0 : q0 + BW], gamma, xs[br][:, 1, q0 : q0 + BW],
                    ALU.mult, ALU.add,
                )

            def store_o(br, od):
                nc.sync.dma_start(od[0:128, :], out_e[br][:, 0, 64:1088])
                nc.sync.dma_start(od[128:256, :], out_e[br][:, 1, 64:1088])

            # ---- streaming pair pipeline across all 6 blocks ----
            ORDER = [(0, 0), (1, 0), (0, 1), (1, 1), (0, 2), (1, 2)]
            chase_map = {}  # (bi, p) -> thunks emitted before that pair's S
            post_map = {}  # (bi, p) -> thunks emitted right after that pair's exp
            for nt in range(8):
                chase_map[(0, 2 * nt)] = [lambda nt=nt: emit_psk(0, nt)]
                chase_map[(1, 2 * nt)] = [lambda nt=nt: emit_psk(1, nt)]
                post_map[(0, 2 * nt)] = [lambda nt=nt: emit_psv(0, nt)]
                post_map[(1, 2 * nt)] = [lambda nt=nt: emit_psv(1, nt)]
            chase_map[(0, 0)].append(lambda: emit_psq(0, 0))
            chase_map[(0, 2)].append(lambda: emit_psq(1, 0))
            post_map[(0, 8)].append(lambda: emit_xb(0))
            post_map[(0, 10)].append(lambda: emit_xb(1))
            chase_map[(1, 4)].append(lambda: emit_psq(0, 1))
            chase_map[(1, 6)].append(lambda: emit_psq(1, 1))

            between_map = {  # (bi, p) -> thunks emitted after that pair's pop
                (1, 2): [lambda: emit_strip(0, 0)],
                (2, 1): [lambda: emit_strip(1, 0), lambda: emit_spad_prep(0),
                         lambda: emit_psq(0, 2)],
                (3, 2): [lambda: emit_psq(1, 2)],
                (4, 1): [lambda: emit_spad_prep(1)],
                (5, 1): [lambda: emit_strip(0, 17)],
                (5, 2): [lambda: store_o(0, o1)],
            }
            # conv slabs 0..4 spread across blocks idx2-idx5 in 6-matmul chunks
            fill = []
            for k, oc in [(0, 0), (0, 1), (1, 0), (1, 1)]:
                fill += conv_slab_thunks(k, oc)
            for i, th in enumerate(fill):
                between_map.setdefault((2, 3 + (i * 12) // len(fill))
                                       if False else (2 + (3 + i) // 14, (3 + i) % 14),
                                       []).append(th)
            fill2 = []
            for k, oc in [(2, 0), (2, 1), (3, 0), (3, 1), (4, 0), (4, 1)]:
                fill2 += conv_slab_thunks(k, oc)
            for i, th in enumerate(fill2):
                between_map.setdefault((4 + (2 + i) // 14, (2 + i) % 14),
                                       []).append(th)

            block_acc = {}
            pend = deque()

            def pop_one():
                bi, br, p, E = pend.popleft()
                if p == 0:
                    block_acc[bi] = (
                        app.tile([128, BW], F32, name="av"),
                        app.tile([128, BW], F32, name="den"),
                    )
                av, den = block_acc[bi]
                nc.tensor.matmul(
                    av[:], vT[br][:, p : p + 1, :, :], E[:],
                    start=(p == 0), stop=(p == 15),
                    perf_mode=DR, skip_group_check=True,
                )
                nc.tensor.matmul(
                    den[:], ones_dr[:], E[:],
                    start=(p == 0), stop=(p == 15),
                    perf_mode=DR, skip_group_check=True,
                )
                if p == 15:
                    br_, blk_ = ORDER[bi]
                    emit_postlude(br_, blk_, av, den)

            for bi, (br, blk) in enumerate(ORDER):
                q0 = blk * BW
                for p in range(16):
                    for th in chase_map.get((bi, p), ()):
                        th()
                    s_t = spool.tile([128, 2, 512], F32, name="s_t")
                    for j in range(2):
                        nc.tensor.matmul(
                            s_t[:, j, 0:BW],
                            k_r[br][:, (2 * p + j) * 128 : (2 * p + j) * 128 + 128],
                            q_r[br][:, q0 : q0 + BW],
                            start=True,
                            stop=True,
                        )
                    E = ep.tile([128, 2, BW], FP8, name="E")
                    nc.scalar.activation(
                        E[:], s_t[:, :, 0:BW], AF.Exp, scale=0.25, bias=negc[:]
                    )
                    pend.append((bi, br, p, E))
                    for th in post_map.get((bi, p), ()):
                        th()
                    if len(pend) > 2:
                        pop_one()
                    for th in between_map.get((bi, p), ()):
                        th()
            while pend:
                pop_one()

            # ---- tail: last conv rows + stores ----
            emit_strip(1, 17)
            emit_spad_prep(2, half=0)
            store_o(1, o2)
            emit_conv_slab(5, 0)
            emit_spad_prep(2, half=1)
            emit_conv_slab(5, 1)
            for k in (6, 7):
                for oc in range(2):
                    emit_conv_slab(k, oc)

    nc.compile()
    return nc


def _prep_inputs(input1, input2, Wq, bq, Wk, bk, Wv, bv, gamma, Wcat, bn_gamma, bn_beta):
    f32 = np.float32
    bf16 = ml_dtypes.bfloat16
    g = f32(np.asarray(gamma).reshape(-1)[0])
    x1 = np.asarray(input1, f32).reshape(B, C, NW)
    x2 = np.asarray(input2, f32).reshape(B, C, NW)
    Wq, Wk, Wv = (np.asarray(w, f32) for w in (Wq, Wk, Wv))
    Wcat = np.asarray(Wcat, f32)

    wqkv = np.zeros((128, 2, 160), f32)
    for cc in range(2):
        wqkv[:, cc, 0:16] = Wq.T[128 * cc : 128 * cc + 128]
        wqkv[:, cc, 16:32] = Wk.T[128 * cc : 128 * cc + 128]
        # column-reversed for the DoubleRowSwInterleave weight layout
        wqkv[:, cc, 32:160] = Wv.T[128 * cc : 128 * cc + 128][:, ::-1]

    # [t, cin, cout]
    Wt = Wcat.transpose(2, 3, 1, 0).reshape(9, 256, 256)
    wcat2 = np.zeros((128, 2, 9, 256), f32)
    for cc in range(2):
        wcat2[:, cc] = Wt[:, 128 * cc : 128 * cc + 128, :].transpose(1, 0, 2)

    small = np.zeros((128, 8), f32)
    small[0:16, 0] = np.asarray(bq, f32)
    small[0:16, 1] = np.asarray(bk, f32)
    small[:, 2] = g * np.asarray(bv, f32)
    bnscale = (np.asarray(bn_gamma, f32) / np.sqrt(f32(1.0) + f32(BN_EPS))).astype(f32)
    bnb = np.asarray(bn_beta, f32)
    small[:, 3] = bnscale[0:128]
    small[:, 4] = bnscale[128:256]
    small[:, 5] = bnb[0:128]
    small[:, 6] = bnb[128:256]

    wqkv_b = wqkv.astype(bf16)
    wcat_b = wcat2.astype(bf16)

    in_maps = []
    for core in range(N_CORES):
        b, rg = core // 4, core % 4
        roll = (16 * rg - 1) * 64  # window col j = image pos (roll + j) mod NW

        rows = np.ones(RE, f32)
        if rg == 0:
            rows[0] = 0.0
        if rg == 3:
            rows[RE - 1] = 0.0
        msk = np.broadcast_to(
            np.repeat(rows, W).reshape(RE, W)[None], (128, RE, W)
        ).astype(bf16)

        in_maps.append(
            {
                "x1d": np.ascontiguousarray(np.roll(x1[b], -roll, axis=1)).astype(bf16),
                "x2d": np.ascontiguousarray(np.roll(x2[b], -roll, axis=1)).astype(bf16),
                "wqkvd": wqkv_b,
                "wcatd": wcat_b,
                "maskd": msk,
                "smalld": small,
            }
        )
    return in_maps


def _assemble(results):
    f32 = np.float32
    feat_sum = np.empty((B, C, H, W), f32)
    out1 = np.empty((B, C, H, W), f32)
    out2 = np.empty((B, C, H, W), f32)
    for core in range(N_CORES):
        b, rg = core // 4, core % 4
        r0 = 16 * rg
        r = results[core]
        out1[b, :, r0 : r0 + 16] = r["o1"].reshape(C, 16, W)
        out2[b, :, r0 : r0 + 16] = r["o2"].reshape(C, 16, W)
        feat_sum[b, :, r0 : r0 + 16] = r["feat"].reshape(C, 16, W)
    return feat_sum, out1, out2


def _get_program(gamma: float):
    if gamma not in _PROG_CACHE:
        _PROG_CACHE[gamma] = _build_program(gamma)
    return _PROG_CACHE[gamma]


def kernel(input1, input2, Wq, bq, Wk, bk, Wv, bv, gamma, Wcat, bn_gamma, bn_beta):
    g = float(np.asarray(gamma).reshape(-1)[0])
    nc = _get_program(g)
    in_maps = _prep_inputs(
        input1, input2, Wq, bq, Wk, bk, Wv, bv, gamma, Wcat, bn_gamma, bn_beta
    )
    res = run_bass_kernel_spmd(nc, in_maps, core_ids=list(range(N_CORES)))
    return _assemble(res.results)


def run_traced(inputs):
    """For test.py: run and return (outputs, exec_time_ns)."""
    g = float(np.asarray(inputs["gamma"]).reshape(-1)[0])
    nc = _get_program(g)
    in_maps = _prep_inputs(**inputs)
    res = run_bass_kernel_spmd(nc, in_maps, core_ids=list(range(N_CORES)))
    return _assemble(res.results), res.exec_time_ns


# revision 21
# speedup vs baseline: 1.0186x; 1.0047x over previous
"""Trainium2 Bass kernel for nn_CrossAtt (dual cross-attention + concat +
residual + 3x3 conv + BN + ReLU), data-parallel over (batch, row-group)
across 8 cores.

Sharding: core i -> batch b = i//4, row-group rg = i%4 (16 output rows).
Host pre-ROLLS each core's x1/x2 along the flattened HW axis so the core's
18-row extended query window (16 rows + 1 halo row each side) sits at a
FIXED column range [0, 1152) of the rolled buffer. Attention is invariant
to the consistent key permutation the roll induces; edge-wrap halo rows are
zeroed by the conv mask, exactly like the out-of-image rows they replace.

Numerics / engine assignment:
 - x, Wq/Wk/Wv, Wcat in bf16 (full PE rate at any free size; halves DMA;
   kills all fp32->fp32r rounding copies).
 - S = k^T q accumulated in fp32 PSUM; exp on ACT with scale=1/4 and
   bias=-2 (range guard), output E in fp8e4.
 - A@V and softmax denominator via fp8 DoubleRow matmuls (two 128-key
   chunks per instruction).
 - Softmax normalization: DVE reciprocal + GPSIMD partition_broadcast
   (no ones-matmul / ACT copy).
 - conv runs as 2-row output slabs interleaved between attention blocks;
   BN+ReLU fused on ACT.
"""

import sys

sys.path.insert(0, "/opt/trn_rl_repo")

from collections import deque

import numpy as np
import ml_dtypes

import concourse.bacc as bacc
import concourse.tile as tile
from concourse import mybir
from concourse.bass_utils import run_bass_kernel_spmd

F32 = mybir.dt.float32
BF16 = mybir.dt.bfloat16
FP8 = mybir.dt.float8e4
AF = mybir.ActivationFunctionType
ALU = mybir.AluOpType
DR = mybir.MatmulPerfMode.DoubleRowSwInterleave

B, C, H, W = 2, 256, 64, 64
NW = H * W  # 4096 key positions
RE = 18  # extended rows per core (16 + halo)
NE = RE * W  # 1152 query positions
D_QK, D_V = 16, 128
N_CORES = 8
BN_EPS = 1e-5
BW = 384  # query block width (3 blocks per branch)
NBLK = NE // BW

_PROG_CACHE: dict = {}


def _build_program(gamma: float):
    nc = bacc.Bacc("TRN2", target_bir_lowering=False, debug=False, num_devices=N_CORES)

    def din(name, shape, dt=F32):
        return nc.dram_tensor(name, shape, dt, kind="ExternalInput").ap()

    def dout(name, shape):
        return nc.dram_tensor(name, shape, F32, kind="ExternalOutput").ap()

    x1d = din("x1d", [C, NW], BF16)
    x2d = din("x2d", [C, NW], BF16)
    wqkvd = din("wqkvd", [128, 2, 160], BF16)
    wcatd = din("wcatd", [128, 2, 9, 256], BF16)
    maskd = din("maskd", [128, RE, W], BF16)
    smalld = din("smalld", [128, 8])
    o1 = dout("o1", [C, 1024])
    o2 = dout("o2", [C, 1024])
    feat = dout("feat", [C, 1024])

    with tile.TileContext(nc) as tc:
        with (
            tc.tile_pool(name="constp", bufs=1) as constp,
            tc.tile_pool(name="projp", bufs=1) as projp,
            tc.tile_pool(name="outp", bufs=1) as outp,
            tc.tile_pool(name="ep", bufs=4) as ep,
            tc.tile_pool(name="wkp", bufs=3) as wkp,
            tc.tile_pool(name="spool", bufs=2, space="PSUM") as spool,
            tc.tile_pool(name="app", bufs=1, space="PSUM") as app,
            tc.tile_pool(name="pjp", bufs=1, space="PSUM") as pjp,
        ):
            # ---- persistent SBUF tiles ----
            x1 = constp.tile([128, 2, NW], BF16, name="x1")
            x2 = constp.tile([128, 2, NW], BF16, name="x2")
            wqkv = constp.tile([128, 2, 160], BF16, name="wqkv")
            wcat = constp.tile([128, 2, 9, 256], BF16, name="wcat")
            mask = constp.tile([128, RE, W], BF16, name="mask")
            small = constp.tile([128, 8], F32, name="small")
            wqk = wqkv[:, :, 0:32]
            wv = wqkv[:, :, 32:160]
            bqk = small[0:16, 0:2]
            bvg = small[:, 2:3]
            bn = small[:, 3:7]
            ones_dr = constp.tile([128, 2, 128], FP8, name="ones_dr")
            negc = constp.tile([128, 1], F32, name="negc")

            k_r = [projp.tile([16, NW], BF16, name=f"k_r{i}") for i in range(2)]
            q_r = [projp.tile([16, NE], BF16, name=f"q_r{i}") for i in range(2)]
            vT = [projp.tile([128, 16, 128, 2], FP8, name=f"vT{i}") for i in range(2)]
            xb = [projp.tile([128, NE], BF16, name=f"xb{i}") for i in range(2)]
            out_e = [outp.tile([128, 2, NE], F32, name=f"out_e{i}") for i in range(2)]
            spad = outp.tile([128, 2, RE, W + 2], BF16, name="spad")

            xs = [x1, x2]  # self input per branch
            xo = [x2, x1]  # other input per branch

            # ---- DMA issue: sync queue carries cc0, scalar cc1; x2 leads ----
            nc.sync.dma_start(small[:], smalld[:])
            nc.scalar.dma_start(wqkv[:], wqkvd[:])
            QW = 1024
            xq_order = [(x2, x2d, 0), (x2, x2d, 1), (x1, x1d, 0), (x2, x2d, 2),
                        (x1, x1d, 1), (x2, x2d, 3), (x1, x1d, 2), (x1, x1d, 3)]
            for xt, xd, q in xq_order:
                nc.sync.dma_start(
                    xt[:, 0, QW * q : QW * q + QW], xd[0:128, QW * q : QW * q + QW]
                )
                nc.scalar.dma_start(
                    xt[:, 1, QW * q : QW * q + QW], xd[128:256, QW * q : QW * q + QW]
                )
            # late-needed tensors queued last so they can't delay the x loads
            nc.sync.dma_start(mask[:], maskd[:])
            nc.scalar.dma_start(wcat[:], wcatd[:])
            nc.gpsimd.memset(ones_dr[:], 1.0)
            nc.gpsimd.memset(negc[:], -3.5)
            nc.gpsimd.memset(spad[:], 0.0)

            # ---- emission helpers ----
            def emit_psk(br, nt):
                """k projection for 512 keys: k of branch br comes from x_other."""
                psk = pjp.tile([16, 512], F32, name="psk")
                c0 = nt * 512
                for cc in range(2):
                    nc.tensor.matmul(
                        psk[:],
                        wqk[:, cc, 16:32],
                        xo[br][:, cc, c0 : c0 + 512],
                        start=(cc == 0),
                        stop=(cc == 1),
                    )
                nc.vector.tensor_scalar_add(k_r[br][:, c0 : c0 + 512], psk[:], bqk[:, 1:2])

            def emit_psv(br, nt):
                """v projection for 4 key chunks -> vT[br][:, 4nt:4nt+4, :] fp8."""
                psv = pjp.tile([128, 512], F32, name="psv")
                for s4 in range(4):
                    p0 = nt * 512 + s4 * 128
                    for cc in range(2):
                        nc.tensor.matmul(
                            psv[:, s4 * 128 : s4 * 128 + 128],
                            xs[br][:, cc, p0 : p0 + 128],
                            wv[:, cc, :],
                            start=(cc == 0),
                            stop=(cc == 1),
                            skip_group_check=True,
                        )
                for h in range(2):
                    for sub in range(2):
                        nc.vector.tensor_copy(
                            vT[br][:, 2 * nt + h : 2 * nt + h + 1, :, sub : sub + 1],
                            psv[:, (2 * h + sub) * 128 : (2 * h + sub + 1) * 128],
                        )

            def emit_psq(br, blk):
                psq = pjp.tile([16, BW], F32, name="psq", tag="psk")
                q0 = blk * BW
                for cc in range(2):
                    nc.tensor.matmul(
                        psq[:],
                        wqk[:, cc, 0:16],
                        xs[br][:, cc, q0 : q0 + BW],
                        start=(cc == 0),
                        stop=(cc == 1),
                    )
                nc.vector.tensor_scalar_add(q_r[br][:, q0 : q0 + BW], psq[:], bqk[:, 0:1])

            def emit_xb(br):
                # x_self low channels + gamma*bv (residual+bias base for concat half)
                nc.vector.tensor_scalar_add(xb[br][:], xs[br][:, 0, 0:NE], bvg[:])

            # conv slabs: slab k covers output window rows {2k+1, 2k+2}.
            # Emitted as chunks of 6 matmuls so interleaving into the attention
            # pair stream never stalls the exp cadence.
            def conv_slab_thunks(k, oc):
                psy_box = {}

                def mm_chunk(ci):
                    def run():
                        if ci == 0:
                            psy_box["t"] = pjp.tile(
                                [128, 128], F32, name="psy",
                                tag=("psk" if oc == 0 else "psv"),
                            )
                        psy = psy_box["t"]
                        for t in range(3 * ci, 3 * ci + 3):
                            dy, dx = t // 3, t % 3
                            for cc in range(2):
                                nc.tensor.matmul(
                                    psy[:],
                                    wcat[:, cc, t, oc * 128 : oc * 128 + 128],
                                    spad[:, cc, 2 * k + dy : 2 * k + dy + 2, dx : dx + 64],
                                    start=(t == 0 and cc == 0),
                                    stop=(t == 8 and cc == 1),
                                    skip_group_check=True,
                                )
                    return run

                def finish():
                    psy = psy_box["t"]
                    fs = wkp.tile([128, 128], F32, name="fs")
                    nc.scalar.activation(
                        fs[:], psy[:], AF.Relu,
                        bias=bn[:, 2 + oc : 3 + oc], scale=bn[:, oc : oc + 1],
                    )
                    nc.sync.dma_start(
                        feat[128 * oc : 128 * oc + 128, 128 * k : 128 * k + 128], fs[:]
                    )

                return [mm_chunk(0), mm_chunk(1), mm_chunk(2), finish]

            def emit_conv_slab(k, oc):
                for th in conv_slab_thunks(k, oc):
                    th()

            def emit_spad_prep(b, half=None):
                """spad rows 6b..6b+6 = out1+out2 (edge rows pre-masked in
                out_e by emit_strip). half=0/1 emits 3-row halves."""
                rr = {None: (0, 6), 0: (0, 3), 1: (3, 6)}[half]
                for cc in range(2):
                    r0, q0 = 6 * b + rr[0], (6 * b + rr[0]) * W
                    n = (rr[1] - rr[0]) * W
                    nc.vector.tensor_add(
                        spad[:, cc, r0 : r0 + rr[1] - rr[0], 1:65],
                        out_e[0][:, cc, q0 : q0 + n],
                        out_e[1][:, cc, q0 : q0 + n],
                    )

            def emit_strip(br, row):
                # zero the wrap-garbage halo row of out_e (only rg edge cores
                # have a 0-row in mask; interior cores multiply by ones)
                q0 = row * W
                for cc in range(2):
                    nc.vector.tensor_tensor(
                        out_e[br][:, cc, q0 : q0 + W],
                        out_e[br][:, cc, q0 : q0 + W],
                        mask[:, row, :],
                        ALU.mult,
                    )

            def emit_postlude(br, blk, av, den):
                # normalization + residual/concat epilogue
                q0 = blk * BW
                rb = wkp.tile([128, BW], F32, name="rb")
                nc.vector.reciprocal(rb[:], den[:])
                tmp = wkp.tile([128, BW], F32, name="tmp")
                nc.vector.tensor_tensor(tmp[:], av[:], rb[:], ALU.mult)
                nc.vector.scalar_tensor_tensor(
                    out_e[br][:, 0, q0 : q0 + BW], tmp[:], gamma, xb[br][:, q0 : q0 + BW],
                    ALU.mult, ALU.add,
                )
                nc.vector.scalar_tensor_tensor(
                    out_e[br][:, 1, q0 : q0 + BW],
                    xs[br][:, 0, q0 : q0 + BW], gamma, xs[br][:, 1, q0 : q0 + BW],
                    ALU.mult, ALU.add,
                )

            def store_o(br, od):
                nc.sync.dma_start(od[0:128, :], out_e[br][:, 0, 64:1088])
                nc.sync.dma_start(od[128:256, :], out_e[br][:, 1, 64:1088])

            # ---- streaming pair pipeline across all 6 blocks ----
            ORDER = [(0, 0), (1, 0), (0, 1), (1, 1), (0, 2), (1, 2)]
            chase_map = {}  # (bi, p) -> thunks emitted before that pair's S
            post_map = {}  # (bi, p) -> thunks emitted right after that pair's exp
            for nt in range(8):
                chase_map[(0, 2 * nt)] = [lambda nt=nt: emit_psk(0, nt)]
                chase_map[(1, 2 * nt)] = [lambda nt=nt: emit_psk(1, nt)]
                post_map[(0, 2 * nt + 1)] = [lambda nt=nt: emit_psv(0, nt)]
                post_map[(1, 2 * nt + 1)] = [lambda nt=nt: emit_psv(1, nt)]
            chase_map[(0, 0)].append(lambda: emit_psq(0, 0))
            chase_map[(0, 2)].append(lambda: emit_psq(1, 0))
            post_map.setdefault((0, 8), []).append(lambda: emit_xb(0))
            post_map.setdefault((0, 10), []).append(lambda: emit_xb(1))
            chase_map[(1, 4)].append(lambda: emit_psq(0, 1))
            chase_map[(1, 6)].append(lambda: emit_psq(1, 1))

            between_map = {  # (bi, p) -> thunks emitted after that pair's pop
                (1, 2): [lambda: emit_strip(0, 0)],
                (2, 1): [lambda: emit_strip(1, 0), lambda: emit_spad_prep(0),
                         lambda: emit_psq(0, 2)],
                (3, 2): [lambda: emit_psq(1, 2)],
                (4, 1): [lambda: emit_spad_prep(1)],
                (5, 1): [lambda: emit_strip(0, 17)],
                (5, 2): [lambda: store_o(0, o1)],
            }
            # conv slabs 0..4 spread across blocks idx2-idx5 in 6-matmul chunks
            fill = []
            for k, oc in [(0, 0), (0, 1), (1, 0), (1, 1)]:
                fill += conv_slab_thunks(k, oc)
            for i, th in enumerate(fill):
                between_map.setdefault((2, 3 + (i * 12) // len(fill))
                                       if False else (2 + (3 + i) // 14, (3 + i) % 14),
                                       []).append(th)
            fill2 = []
            for k, oc in [(2, 0), (2, 1), (3, 0), (3, 1), (4, 0), (4, 1)]:
                fill2 += conv_slab_thunks(k, oc)
            for i, th in enumerate(fill2):
                between_map.setdefault((4 + (2 + i) // 14, (2 + i) % 14),
                                       []).append(th)

            block_acc = {}
            pend = deque()

            def pop_one():
                bi, br, p, E = pend.popleft()
                if p == 0:
                    block_acc[bi] = (
                        app.tile([128, BW], F32, name="av"),
                        app.tile([128, BW], F32, name="den"),
                    )
                av, den = block_acc[bi]
                nc.tensor.matmul(
                    av[:], vT[br][:, p : p + 1, :, :], E[:],
                    start=(p == 0), stop=(p == 15),
                    perf_mode=DR, skip_group_check=True,
                )
                nc.tensor.matmul(
                    den[:], ones_dr[:], E[:],
                    start=(p == 0), stop=(p == 15),
                    perf_mode=DR, skip_group_check=True,
                )
                if p == 15:
                    br_, blk_ = ORDER[bi]
                    emit_postlude(br_, blk_, av, den)

            for bi, (br, blk) in enumerate(ORDER):
                q0 = blk * BW
                for p in range(16):
                    for th in chase_map.get((bi, p), ()):
                        th()
                    s_t = spool.tile([128, 2, 512], F32, name="s_t")
                    for j in range(2):
                        nc.tensor.matmul(
                            s_t[:, j, 0:BW],
                            k_r[br][:, (2 * p + j) * 128 : (2 * p + j) * 128 + 128],
                            q_r[br][:, q0 : q0 + BW],
                            start=True,
                            stop=True,
                        )
                    E = ep.tile([128, 2, BW], FP8, name="E")
                    nc.scalar.activation(
                        E[:], s_t[:, :, 0:BW], AF.Exp, scale=0.25, bias=negc[:]
                    )
                    pend.append((bi, br, p, E))
                    for th in post_map.get((bi, p), ()):
                        th()
                    if len(pend) > 2:
                        pop_one()
                    for th in between_map.get((bi, p), ()):
                        th()
            while pend:
                pop_one()

            # ---- tail: last conv rows + stores ----
            emit_strip(1, 17)
            emit_spad_prep(2, half=0)
            store_o(1, o2)
            emit_conv_slab(5, 0)
            emit_spad_prep(2, half=1)
            emit_conv_slab(5, 1)
            for k in (6, 7):
                for oc in range(2):
                    emit_conv_slab(k, oc)

    nc.compile()
    return nc


def _prep_inputs(input1, input2, Wq, bq, Wk, bk, Wv, bv, gamma, Wcat, bn_gamma, bn_beta):
    f32 = np.float32
    bf16 = ml_dtypes.bfloat16
    g = f32(np.asarray(gamma).reshape(-1)[0])
    x1 = np.asarray(input1, f32).reshape(B, C, NW)
    x2 = np.asarray(input2, f32).reshape(B, C, NW)
    Wq, Wk, Wv = (np.asarray(w, f32) for w in (Wq, Wk, Wv))
    Wcat = np.asarray(Wcat, f32)

    wqkv = np.zeros((128, 2, 160), f32)
    for cc in range(2):
        wqkv[:, cc, 0:16] = Wq.T[128 * cc : 128 * cc + 128]
        wqkv[:, cc, 16:32] = Wk.T[128 * cc : 128 * cc + 128]
        # column-reversed for the DoubleRowSwInterleave weight layout
        wqkv[:, cc, 32:160] = Wv.T[128 * cc : 128 * cc + 128][:, ::-1]

    # [t, cin, cout]
    Wt = Wcat.transpose(2, 3, 1, 0).reshape(9, 256, 256)
    wcat2 = np.zeros((128, 2, 9, 256), f32)
    for cc in range(2):
        wcat2[:, cc] = Wt[:, 128 * cc : 128 * cc + 128, :].transpose(1, 0, 2)

    small = np.zeros((128, 8), f32)
    small[0:16, 0] = np.asarray(bq, f32)
    small[0:16, 1] = np.asarray(bk, f32)
    small[:, 2] = g * np.asarray(bv, f32)
    bnscale = (np.asarray(bn_gamma, f32) / np.sqrt(f32(1.0) + f32(BN_EPS))).astype(f32)
    bnb = np.asarray(bn_beta, f32)
    small[:, 3] = bnscale[0:128]
    small[:, 4] = bnscale[128:256]
    small[:, 5] = bnb[0:128]
    small[:, 6] = bnb[128:256]

    wqkv_b = wqkv.astype(bf16)
    wcat_b = wcat2.astype(bf16)

    in_maps = []
    for core in range(N_CORES):
        b, rg = core // 4, core % 4
        roll = (16 * rg - 1) * 64  # window col j = image pos (roll + j) mod NW

        rows = np.ones(RE, f32)
        if rg == 0:
            rows[0] = 0.0
        if rg == 3:
            rows[RE - 1] = 0.0
        msk = np.broadcast_to(
            np.repeat(rows, W).reshape(RE, W)[None], (128, RE, W)
        ).astype(bf16)

        in_maps.append(
            {
                "x1d": np.ascontiguousarray(np.roll(x1[b], -roll, axis=1)).astype(bf16),
                "x2d": np.ascontiguousarray(np.roll(x2[b], -roll, axis=1)).astype(bf16),
                "wqkvd": wqkv_b,
                "wcatd": wcat_b,
                "maskd": msk,
                "smalld": small,
            }
        )
    return in_maps


def _assemble(results):
    f32 = np.float32
    feat_sum = np.empty((B, C, H, W), f32)
    out1 = np.empty((B, C, H, W), f32)
    out2 = np.empty((B, C, H, W), f32)
    for core in range(N_CORES):
        b, rg = core // 4, core % 4
        r0 = 16 * rg
        r = results[core]
        out1[b, :, r0 : r0 + 16] = r["o1"].reshape(C, 16, W)
        out2[b, :, r0 : r0 + 16] = r["o2"].reshape(C, 16, W)
        feat_sum[b, :, r0 : r0 + 16] = r["feat"].reshape(C, 16, W)
    return feat_sum, out1, out2


def _get_program(gamma: float):
    if gamma not in _PROG_CACHE:
        _PROG_CACHE[gamma] = _build_program(gamma)
    return _PROG_CACHE[gamma]


def kernel(input1, input2, Wq, bq, Wk, bk, Wv, bv, gamma, Wcat, bn_gamma, bn_beta):
    g = float(np.asarray(gamma).reshape(-1)[0])
    nc = _get_program(g)
    in_maps = _prep_inputs(
        input1, input2, Wq, bq, Wk, bk, Wv, bv, gamma, Wcat, bn_gamma, bn_beta
    )
    res = run_bass_kernel_spmd(nc, in_maps, core_ids=list(range(N_CORES)))
    return _assemble(res.results)


def run_traced(inputs):
    """For test.py: run and return (outputs, exec_time_ns)."""
    g = float(np.asarray(inputs["gamma"]).reshape(-1)[0])
    nc = _get_program(g)
    in_maps = _prep_inputs(**inputs)
    res = run_bass_kernel_spmd(nc, in_maps, core_ids=list(range(N_CORES)))
    return _assemble(res.results), res.exec_time_ns


# revision 22
# speedup vs baseline: 1.0275x; 1.0088x over previous
"""Trainium2 Bass kernel for nn_CrossAtt (dual cross-attention + concat +
residual + 3x3 conv + BN + ReLU), data-parallel over (batch, row-group)
across 8 cores.

Sharding: core i -> batch b = i//4, row-group rg = i%4 (16 output rows).
Host pre-ROLLS each core's x1/x2 along the flattened HW axis so the core's
18-row extended query window (16 rows + 1 halo row each side) sits at a
FIXED column range [0, 1152) of the rolled buffer. Attention is invariant
to the consistent key permutation the roll induces; edge-wrap halo rows are
zeroed by the conv mask, exactly like the out-of-image rows they replace.

Numerics / engine assignment:
 - x, Wq/Wk/Wv, Wcat in bf16 (full PE rate at any free size; halves DMA;
   kills all fp32->fp32r rounding copies).
 - S = k^T q accumulated in fp32 PSUM; exp on ACT with scale=1/4 and
   bias=-2 (range guard), output E in fp8e4.
 - A@V and softmax denominator via fp8 DoubleRow matmuls (two 128-key
   chunks per instruction).
 - Softmax normalization: DVE reciprocal + GPSIMD partition_broadcast
   (no ones-matmul / ACT copy).
 - conv runs as 2-row output slabs interleaved between attention blocks;
   BN+ReLU fused on ACT.
"""

import sys

sys.path.insert(0, "/opt/trn_rl_repo")

from collections import deque

import numpy as np
import ml_dtypes

import concourse.bacc as bacc
import concourse.tile as tile
from concourse import mybir
from concourse.bass_utils import run_bass_kernel_spmd

F32 = mybir.dt.float32
BF16 = mybir.dt.bfloat16
FP8 = mybir.dt.float8e4
AF = mybir.ActivationFunctionType
ALU = mybir.AluOpType
DR = mybir.MatmulPerfMode.DoubleRowSwInterleave

B, C, H, W = 2, 256, 64, 64
NW = H * W  # 4096 key positions
RE = 18  # extended rows per core (16 + halo)
NE = RE * W  # 1152 query positions
D_QK, D_V = 16, 128
N_CORES = 8
BN_EPS = 1e-5
BW = 384  # query block width (3 blocks per branch)
NBLK = NE // BW

_PROG_CACHE: dict = {}


def _build_program(gamma: float):
    nc = bacc.Bacc("TRN2", target_bir_lowering=False, debug=False, num_devices=N_CORES)

    def din(name, shape, dt=F32):
        return nc.dram_tensor(name, shape, dt, kind="ExternalInput").ap()

    def dout(name, shape):
        return nc.dram_tensor(name, shape, F32, kind="ExternalOutput").ap()

    x1d = din("x1d", [C, NW], BF16)
    x2d = din("x2d", [C, NW], BF16)
    wqkvd = din("wqkvd", [128, 2, 160], BF16)
    wcatd = din("wcatd", [128, 2, 9, 256], BF16)
    maskd = din("maskd", [128, RE, W], BF16)
    smalld = din("smalld", [128, 8])
    o1 = dout("o1", [C, 1024])
    o2 = dout("o2", [C, 1024])
    feat = dout("feat", [C, 1024])

    with tile.TileContext(nc) as tc:
        with (
            tc.tile_pool(name="constp", bufs=1) as constp,
            tc.tile_pool(name="projp", bufs=1) as projp,
            tc.tile_pool(name="outp", bufs=1) as outp,
            tc.tile_pool(name="ep", bufs=4) as ep,
            tc.tile_pool(name="wkp", bufs=3) as wkp,
            tc.tile_pool(name="spool", bufs=2, space="PSUM") as spool,
            tc.tile_pool(name="app", bufs=1, space="PSUM") as app,
            tc.tile_pool(name="pjp", bufs=1, space="PSUM") as pjp,
        ):
            # ---- persistent SBUF tiles ----
            x1 = constp.tile([128, 2, NW], BF16, name="x1")
            x2 = constp.tile([128, 2, NW], BF16, name="x2")
            wqkv = constp.tile([128, 2, 160], BF16, name="wqkv")
            wcat = constp.tile([128, 2, 9, 256], BF16, name="wcat")
            mask = constp.tile([128, RE, W], BF16, name="mask")
            small = constp.tile([128, 8], F32, name="small")
            wqk = wqkv[:, :, 0:32]
            wv = wqkv[:, :, 32:160]
            bqk = small[0:16, 0:2]
            bvg = small[:, 2:3]
            bn = small[:, 3:7]
            ones_dr = constp.tile([128, 2, 128], FP8, name="ones_dr")
            negc = constp.tile([128, 1], F32, name="negc")

            k_r = [projp.tile([16, NW], BF16, name=f"k_r{i}") for i in range(2)]
            q_r = [projp.tile([16, NE], BF16, name=f"q_r{i}") for i in range(2)]
            vT = [projp.tile([128, 16, 128, 2], FP8, name=f"vT{i}") for i in range(2)]
            xb = [projp.tile([128, NE], BF16, name=f"xb{i}") for i in range(2)]
            out_e = [outp.tile([128, 2, NE], F32, name=f"out_e{i}") for i in range(2)]
            spad = outp.tile([128, 2, RE, W + 2], BF16, name="spad")

            xs = [x1, x2]  # self input per branch
            xo = [x2, x1]  # other input per branch

            # ---- DMA issue: sync queue carries cc0, scalar cc1; x2 leads ----
            nc.sync.dma_start(small[:], smalld[:])
            nc.scalar.dma_start(wqkv[:], wqkvd[:])
            QW = 1024
            xq_order = [(x2, x2d, 0), (x2, x2d, 1), (x1, x1d, 0), (x2, x2d, 2),
                        (x1, x1d, 1), (x2, x2d, 3), (x1, x1d, 2), (x1, x1d, 3)]
            for xt, xd, q in xq_order:
                nc.sync.dma_start(
                    xt[:, 0, QW * q : QW * q + QW], xd[0:128, QW * q : QW * q + QW]
                )
                nc.scalar.dma_start(
                    xt[:, 1, QW * q : QW * q + QW], xd[128:256, QW * q : QW * q + QW]
                )
            # late-needed tensors queued last so they can't delay the x loads
            nc.sync.dma_start(mask[:], maskd[:])
            nc.scalar.dma_start(wcat[:], wcatd[:])
            nc.gpsimd.memset(ones_dr[:], 1.0)
            nc.gpsimd.memset(negc[:], -3.5)
            nc.gpsimd.memset(spad[:], 0.0)

            # ---- emission helpers ----
            def emit_psk(br, nt):
                """k projection for 512 keys: k of branch br comes from x_other."""
                psk = pjp.tile([16, 512], F32, name="psk")
                c0 = nt * 512
                for cc in range(2):
                    nc.tensor.matmul(
                        psk[:],
                        wqk[:, cc, 16:32],
                        xo[br][:, cc, c0 : c0 + 512],
                        start=(cc == 0),
                        stop=(cc == 1),
                    )
                nc.vector.tensor_scalar_add(k_r[br][:, c0 : c0 + 512], psk[:], bqk[:, 1:2])

            def emit_psv(br, nt):
                """v projection for 4 key chunks -> vT[br][:, 4nt:4nt+4, :] fp8."""
                psv = pjp.tile([128, 512], F32, name="psv")
                for s4 in range(4):
                    p0 = nt * 512 + s4 * 128
                    for cc in range(2):
                        nc.tensor.matmul(
                            psv[:, s4 * 128 : s4 * 128 + 128],
                            xs[br][:, cc, p0 : p0 + 128],
                            wv[:, cc, :],
                            start=(cc == 0),
                            stop=(cc == 1),
                            skip_group_check=True,
                        )
                for h in range(2):
                    for sub in range(2):
                        nc.vector.tensor_copy(
                            vT[br][:, 2 * nt + h : 2 * nt + h + 1, :, sub : sub + 1],
                            psv[:, (2 * h + sub) * 128 : (2 * h + sub + 1) * 128],
                        )

            def emit_psq(br, blk):
                psq = pjp.tile([16, BW], F32, name="psq", tag="psk")
                q0 = blk * BW
                for cc in range(2):
                    nc.tensor.matmul(
                        psq[:],
                        wqk[:, cc, 0:16],
                        xs[br][:, cc, q0 : q0 + BW],
                        start=(cc == 0),
                        stop=(cc == 1),
                    )
                nc.vector.tensor_scalar_add(q_r[br][:, q0 : q0 + BW], psq[:], bqk[:, 0:1])

            def emit_xb(br):
                # x_self low channels + gamma*bv (residual+bias base for concat half)
                nc.vector.tensor_scalar_add(xb[br][:], xs[br][:, 0, 0:NE], bvg[:])

            # conv slabs: slab k covers output window rows {2k+1, 2k+2}.
            # Emitted as chunks of 6 matmuls so interleaving into the attention
            # pair stream never stalls the exp cadence.
            def conv_slab_thunks(k, oc):
                psy_box = {}

                def mm_chunk(ci):
                    def run():
                        if ci == 0:
                            psy_box["t"] = pjp.tile(
                                [128, 128], F32, name="psy",
                                tag=("psk" if oc == 0 else "psv"),
                            )
                        psy = psy_box["t"]
                        for t in range(3 * ci, 3 * ci + 3):
                            dy, dx = t // 3, t % 3
                            for cc in range(2):
                                nc.tensor.matmul(
                                    psy[:],
                                    wcat[:, cc, t, oc * 128 : oc * 128 + 128],
                                    spad[:, cc, 2 * k + dy : 2 * k + dy + 2, dx : dx + 64],
                                    start=(t == 0 and cc == 0),
                                    stop=(t == 8 and cc == 1),
                                    skip_group_check=True,
                                )
                    return run

                def finish():
                    psy = psy_box["t"]
                    fs = wkp.tile([128, 128], F32, name="fs")
                    nc.scalar.activation(
                        fs[:], psy[:], AF.Relu,
                        bias=bn[:, 2 + oc : 3 + oc], scale=bn[:, oc : oc + 1],
                    )
                    nc.sync.dma_start(
                        feat[128 * oc : 128 * oc + 128, 128 * k : 128 * k + 128], fs[:]
                    )

                return [mm_chunk(0), mm_chunk(1), mm_chunk(2), finish]

            def emit_conv_slab(k, oc):
                for th in conv_slab_thunks(k, oc):
                    th()

            def emit_spad_prep(b, half=None):
                """spad rows 6b..6b+6 = out1+out2 (edge rows pre-masked in
                out_e by emit_strip). half=0/1 emits 3-row halves."""
                rr = {None: (0, 6), 0: (0, 3), 1: (3, 6)}[half]
                for cc in range(2):
                    r0, q0 = 6 * b + rr[0], (6 * b + rr[0]) * W
                    n = (rr[1] - rr[0]) * W
                    nc.vector.tensor_add(
                        spad[:, cc, r0 : r0 + rr[1] - rr[0], 1:65],
                        out_e[0][:, cc, q0 : q0 + n],
                        out_e[1][:, cc, q0 : q0 + n],
                    )

            def emit_strip(br, row):
                # zero the wrap-garbage halo row of out_e (only rg edge cores
                # have a 0-row in mask; interior cores multiply by ones)
                q0 = row * W
                for cc in range(2):
                    nc.vector.tensor_tensor(
                        out_e[br][:, cc, q0 : q0 + W],
                        out_e[br][:, cc, q0 : q0 + W],
                        mask[:, row, :],
                        ALU.mult,
                    )

            def emit_postlude(br, blk, av, den):
                # normalization + residual/concat epilogue
                q0 = blk * BW
                rb = wkp.tile([128, BW], F32, name="rb")
                nc.vector.reciprocal(rb[:], den[:])
                tmp = wkp.tile([128, BW], F32, name="tmp")
                nc.vector.tensor_tensor(tmp[:], av[:], rb[:], ALU.mult)
                nc.vector.scalar_tensor_tensor(
                    out_e[br][:, 0, q0 : q0 + BW], tmp[:], gamma, xb[br][:, q0 : q0 + BW],
                    ALU.mult, ALU.add,
                )
                nc.vector.scalar_tensor_tensor(
                    out_e[br][:, 1, q0 : q0 + BW],
                    xs[br][:, 0, q0 : q0 + BW], gamma, xs[br][:, 1, q0 : q0 + BW],
                    ALU.mult, ALU.add,
                )

            def store_o(br, od):
                nc.sync.dma_start(od[0:128, :], out_e[br][:, 0, 64:1088])
                nc.sync.dma_start(od[128:256, :], out_e[br][:, 1, 64:1088])

            # ---- streaming pair pipeline across all 6 blocks ----
            ORDER = [(0, 0), (1, 0), (0, 1), (1, 1), (0, 2), (1, 2)]
            chase_map = {}  # (bi, p) -> thunks emitted before that pair's S
            post_map = {}  # (bi, p) -> thunks emitted right after that pair's exp
            for nt in range(8):
                chase_map[(0, 2 * nt)] = [
                    lambda nt=nt: emit_psk(0, nt), lambda nt=nt: emit_psv(0, nt)]
                chase_map[(1, 2 * nt)] = [
                    lambda nt=nt: emit_psk(1, nt), lambda nt=nt: emit_psv(1, nt)]
            chase_map[(0, 0)].append(lambda: emit_psq(0, 0))
            chase_map[(0, 2)].append(lambda: emit_psq(1, 0))
            post_map.setdefault((0, 8), []).append(lambda: emit_xb(0))
            post_map.setdefault((0, 10), []).append(lambda: emit_xb(1))
            chase_map[(1, 4)].append(lambda: emit_psq(0, 1))
            chase_map[(1, 6)].append(lambda: emit_psq(1, 1))

            between_map = {  # (bi, p) -> thunks emitted after that pair's pop
                (1, 2): [lambda: emit_strip(0, 0)],
                (2, 1): [lambda: emit_strip(1, 0), lambda: emit_spad_prep(0),
                         lambda: emit_psq(0, 2)],
                (3, 2): [lambda: emit_psq(1, 2)],
                (4, 1): [lambda: emit_spad_prep(1)],
                (5, 1): [lambda: emit_strip(0, 17)],
                (5, 2): [lambda: store_o(0, o1)],
            }
            # conv slabs 0..4 spread across blocks idx2-idx5 in 6-matmul chunks
            fill = []
            for k, oc in [(0, 0), (0, 1), (1, 0), (1, 1)]:
                fill += conv_slab_thunks(k, oc)
            for i, th in enumerate(fill):
                between_map.setdefault((2, 3 + (i * 12) // len(fill))
                                       if False else (2 + (3 + i) // 14, (3 + i) % 14),
                                       []).append(th)
            fill2 = []
            for k, oc in [(2, 0), (2, 1), (3, 0), (3, 1), (4, 0), (4, 1)]:
                fill2 += conv_slab_thunks(k, oc)
            for i, th in enumerate(fill2):
                between_map.setdefault((4 + (2 + i) // 14, (2 + i) % 14),
                                       []).append(th)

            block_acc = {}
            pend = deque()

            def pop_one():
                bi, br, p, E = pend.popleft()
                if p == 0:
                    block_acc[bi] = (
                        app.tile([128, BW], F32, name="av"),
                        app.tile([128, BW], F32, name="den"),
                    )
                av, den = block_acc[bi]
                nc.tensor.matmul(
                    av[:], vT[br][:, p : p + 1, :, :], E[:],
                    start=(p == 0), stop=(p == 15),
                    perf_mode=DR, skip_group_check=True,
                )
                nc.tensor.matmul(
                    den[:], ones_dr[:], E[:],
                    start=(p == 0), stop=(p == 15),
                    perf_mode=DR, skip_group_check=True,
                )
                if p == 15:
                    br_, blk_ = ORDER[bi]
                    emit_postlude(br_, blk_, av, den)

            for bi, (br, blk) in enumerate(ORDER):
                q0 = blk * BW
                for p in range(16):
                    for th in chase_map.get((bi, p), ()):
                        th()
                    s_t = spool.tile([128, 2, 512], F32, name="s_t")
                    for j in range(2):
                        nc.tensor.matmul(
                            s_t[:, j, 0:BW],
                            k_r[br][:, (2 * p + j) * 128 : (2 * p + j) * 128 + 128],
                            q_r[br][:, q0 : q0 + BW],
                            start=True,
                            stop=True,
                        )
                    E = ep.tile([128, 2, BW], FP8, name="E")
                    nc.scalar.activation(
                        E[:], s_t[:, :, 0:BW], AF.Exp, scale=0.25, bias=negc[:]
                    )
                    pend.append((bi, br, p, E))
                    for th in post_map.get((bi, p), ()):
                        th()
                    if len(pend) > 2:
                        pop_one()
                    for th in between_map.get((bi, p), ()):
                        th()
            while pend:
                pop_one()

            # ---- tail: last conv rows + stores ----
            emit_strip(1, 17)
            emit_spad_prep(2, half=0)
            store_o(1, o2)
            emit_conv_slab(5, 0)
            emit_spad_prep(2, half=1)
            emit_conv_slab(5, 1)
            for k in (6, 7):
                for oc in range(2):
                    emit_conv_slab(k, oc)

    nc.compile()
    return nc


def _prep_inputs(input1, input2, Wq, bq, Wk, bk, Wv, bv, gamma, Wcat, bn_gamma, bn_beta):
    f32 = np.float32
    bf16 = ml_dtypes.bfloat16
    g = f32(np.asarray(gamma).reshape(-1)[0])
    x1 = np.asarray(input1, f32).reshape(B, C, NW)
    x2 = np.asarray(input2, f32).reshape(B, C, NW)
    Wq, Wk, Wv = (np.asarray(w, f32) for w in (Wq, Wk, Wv))
    Wcat = np.asarray(Wcat, f32)

    wqkv = np.zeros((128, 2, 160), f32)
    for cc in range(2):
        wqkv[:, cc, 0:16] = Wq.T[128 * cc : 128 * cc + 128]
        wqkv[:, cc, 16:32] = Wk.T[128 * cc : 128 * cc + 128]
        # column-reversed for the DoubleRowSwInterleave weight layout
        wqkv[:, cc, 32:160] = Wv.T[128 * cc : 128 * cc + 128][:, ::-1]

    # [t, cin, cout]
    Wt = Wcat.transpose(2, 3, 1, 0).reshape(9, 256, 256)
    wcat2 = np.zeros((128, 2, 9, 256), f32)
    for cc in range(2):
        wcat2[:, cc] = Wt[:, 128 * cc : 128 * cc + 128, :].transpose(1, 0, 2)

    small = np.zeros((128, 8), f32)
    small[0:16, 0] = np.asarray(bq, f32)
    small[0:16, 1] = np.asarray(bk, f32)
    small[:, 2] = g * np.asarray(bv, f32)
    bnscale = (np.asarray(bn_gamma, f32) / np.sqrt(f32(1.0) + f32(BN_EPS))).astype(f32)
    bnb = np.asarray(bn_beta, f32)
    small[:, 3] = bnscale[0:128]
    small[:, 4] = bnscale[128:256]
    small[:, 5] = bnb[0:128]
    small[:, 6] = bnb[128:256]

    wqkv_b = wqkv.astype(bf16)
    wcat_b = wcat2.astype(bf16)

    in_maps = []
    for core in range(N_CORES):
        b, rg = core // 4, core % 4
        roll = (16 * rg - 1) * 64  # window col j = image pos (roll + j) mod NW

        rows = np.ones(RE, f32)
        if rg == 0:
            rows[0] = 0.0
        if rg == 3:
            rows[RE - 1] = 0.0
        msk = np.broadcast_to(
            np.repeat(rows, W).reshape(RE, W)[None], (128, RE, W)
        ).astype(bf16)

        in_maps.append(
            {
                "x1d": np.ascontiguousarray(np.roll(x1[b], -roll, axis=1)).astype(bf16),
                "x2d": np.ascontiguousarray(np.roll(x2[b], -roll, axis=1)).astype(bf16),
                "wqkvd": wqkv_b,
                "wcatd": wcat_b,
                "maskd": msk,
                "smalld": small,
            }
        )
    return in_maps


def _assemble(results):
    f32 = np.float32
    feat_sum = np.empty((B, C, H, W), f32)
    out1 = np.empty((B, C, H, W), f32)
    out2 = np.empty((B, C, H, W), f32)
    for core in range(N_CORES):
        b, rg = core // 4, core % 4
        r0 = 16 * rg
        r = results[core]
        out1[b, :, r0 : r0 + 16] = r["o1"].reshape(C, 16, W)
        out2[b, :, r0 : r0 + 16] = r["o2"].reshape(C, 16, W)
        feat_sum[b, :, r0 : r0 + 16] = r["feat"].reshape(C, 16, W)
    return feat_sum, out1, out2


def _get_program(gamma: float):
    if gamma not in _PROG_CACHE:
        _PROG_CACHE[gamma] = _build_program(gamma)
    return _PROG_CACHE[gamma]


def kernel(input1, input2, Wq, bq, Wk, bk, Wv, bv, gamma, Wcat, bn_gamma, bn_beta):
    g = float(np.asarray(gamma).reshape(-1)[0])
    nc = _get_program(g)
    in_maps = _prep_inputs(
        input1, input2, Wq, bq, Wk, bk, Wv, bv, gamma, Wcat, bn_gamma, bn_beta
    )
    res = run_bass_kernel_spmd(nc, in_maps, core_ids=list(range(N_CORES)))
    return _assemble(res.results)


def run_traced(inputs):
    """For test.py: run and return (outputs, exec_time_ns)."""
    g = float(np.asarray(inputs["gamma"]).reshape(-1)[0])
    nc = _get_program(g)
    in_maps = _prep_inputs(**inputs)
    res = run_bass_kernel_spmd(nc, in_maps, core_ids=list(range(N_CORES)))
    return _assemble(res.results), res.exec_time_ns
